# revision 9
# baseline (speedup 1.0000x reference)
"""Causal self-attention (B=4, T=2048, C=1024, H=16) on 8 Trainium2 NeuronCores.

Sharding: core c = 2*b_half... core index = 2*batch + head_group.
Each core handles one batch element (shared by 2 cores) and 8 of the 16 heads
(tensor-parallel split of the c_attn output dim / c_proj input dim).
Each core outputs a partial projection (out^T [C, T]); the host sums the two
head-group partials per batch and adds biases.

Per-core pipeline (all on one NeuronCore, Tile-scheduled):
  A) transpose x -> xT [c, t] via PE-transpose (fp32)
  B) qkT = (W_qk^T x^T) in f32r (full-rate PE), v = x @ W_v natural layout
  C) per head: S^T = k^T q (f32r), P = exp(S/8) fp16 (ACT, fused scale),
     causal mask on diagonal blocks (DVE), U = P^T-stationary @ [v|1] fp16
     (rowsum via ones column), y = U * (1/rowsum) (DVE)
  D) y -> yT via PE-transpose (fp16), out^T = W_p^T y^T (f32r)
"""

import numpy as np

import concourse.bass as bass
import concourse.mybir as mybir
import concourse.tile as tile
from concourse import bacc, bass_utils

B, T, C, H = 4, 2048, 1024, 16
HD = C // H          # 64 head dim
N_CORES = 8
HG = H // 2          # 8 heads per core
CL = HG * HD         # 512 local width of q/k/v
TT = T // 128        # 16 t-tiles
CB = C // 128        # 8 c-tiles
DB = CL // 128       # 4 local-hd tiles

f32 = mybir.dt.float32
f32r = mybir.dt.float32r
f16 = mybir.dt.float16

_PROG_CACHE = {}


def _emit(tc, aps):
    nc = tc.nc
    Exp = mybir.ActivationFunctionType.Exp
    Copy = mybir.ActivationFunctionType.Copy

    x_ap = aps["x"]
    wqk_ap = aps["wqk"]
    wv_ap = aps["wv"]
    wp_ap = aps["wp"]
    bqk_ap = aps["bqk"]
    masks_ap = aps["masks"]
    id32_ap = aps["id32"]
    id16_ap = aps["id16"]
    outT_ap = aps["outT"]

    from contextlib import ExitStack

    with ExitStack() as outer:
        # ---- persistent pools -------------------------------------------------
        const = outer.enter_context(tc.tile_pool(name="const", bufs=1))
        p_qkT = outer.enter_context(tc.tile_pool(name="qkT", bufs=1))
        p_v = outer.enter_context(tc.tile_pool(name="vv", bufs=1))
        p_y = outer.enter_context(tc.tile_pool(name="yy", bufs=1))

        # PSUM pools (8 banks total)
        ps_mm = outer.enter_context(tc.tile_pool(name="ps_mm", bufs=2, space="PSUM"))
        ps_tr = outer.enter_context(tc.tile_pool(name="ps_tr", bufs=2, space="PSUM"))
        ps_sc = outer.enter_context(tc.tile_pool(name="ps_sc", bufs=2, space="PSUM"))
        ps_u = outer.enter_context(tc.tile_pool(name="ps_u", bufs=2, space="PSUM"))

        # constants
        id32 = const.tile([128, 128], f32)
        nc.sync.dma_start(id32[:], id32_ap)
        id16 = const.tile([128, 128], f16)
        nc.sync.dma_start(id16[:], id16_ap)
        masks = const.tile([128, 4, 512], f16)
        nc.sync.dma_start(masks[:], masks_ap)
        bqk = const.tile([128, CB], f32)
        nc.sync.dma_start(bqk[:], bqk_ap.rearrange("co p -> p co"))

        qkT = p_qkT.tile([128, CB, T], f32r)          # q co 0..3, k co 4..7
        vv = p_v.tile([128, TT, HG, HD + 1], f16)     # [j-part, jt, h, d | ones]
        yy = p_y.tile([128, TT, CL], f16)             # [t-part, tt, h*64+d]
        nc.vector.memset(vv[:, :, :, HD : HD + 1], 1.0)

        # ---- scope 1: phases A + B -------------------------------------------
        with ExitStack() as s1:
            p_xload = s1.enter_context(tc.tile_pool(name="xload", bufs=2))
            p_wqk = s1.enter_context(tc.tile_pool(name="wqk", bufs=2))
            p_wv = s1.enter_context(tc.tile_pool(name="wv", bufs=1))
            p_xT = s1.enter_context(tc.tile_pool(name="xT", bufs=1))

            xT = p_xT.tile([128, CB, T], f32r)

            # A: transpose x into xT (f32r rounding on the PSUM->SBUF copy)
            for tt in range(TT):
                xt = p_xload.tile([128, C], f32)
                nc.sync.dma_start(xt[:], x_ap[tt * 128 : (tt + 1) * 128, :])
                for cb in range(CB):
                    pst = ps_tr.tile([128, 128], f32, tag="tr")
                    nc.tensor.transpose(pst[:], xt[:, cb * 128 : (cb + 1) * 128], id32[:])
                    nc.vector.tensor_copy(xT[:, cb, tt * 128 : (tt + 1) * 128], pst[:])

            # B-v: v = x @ Wv in natural [t, d] layout (lhsT = xT stationary)
            wv_r = p_wv.tile([128, CB, CL], f32r)
            for cb in range(CB):
                wv_t = p_xload.tile([128, CL], f32, tag="wstage")
                nc.sync.dma_start(wv_t[:], wv_ap[cb * 128 : (cb + 1) * 128, :])
                nc.vector.tensor_copy(wv_r[:, cb, :], wv_t[:])
            for tt in range(TT):
                ps = ps_mm.tile([128, CL], f32, tag="mm")
                for cb in range(CB):
                    nc.tensor.matmul(
                        ps[:],
                        xT[:, cb, tt * 128 : (tt + 1) * 128],
                        wv_r[:, cb, :],
                        start=(cb == 0),
                        stop=(cb == CB - 1),
                    )
                nc.vector.tensor_copy(
                    vv[:, tt, :, 0:HD], ps.rearrange("p (h d) -> p h d", d=HD)
                )

            # B-qk: qkT = W^T x^T  [c'-part, t-free]
            for co in range(CB):
                wq_r = p_wqk.tile([128, CB, 128], f32r, tag="wqk_r")
                wq_t = p_xload.tile([128, CB, 128], f32, tag="wstage")
                nc.sync.dma_start(wq_t[:], wqk_ap[co].rearrange("(cb p) q -> p cb q", p=128))
                nc.vector.tensor_copy(wq_r[:], wq_t[:])
                for tn in range(4):
                    ps = ps_mm.tile([128, 512], f32, tag="mm")
                    for cb in range(CB):
                        nc.tensor.matmul(
                            ps[:],
                            wq_r[:, cb, :],
                            xT[:, cb, tn * 512 : (tn + 1) * 512],
                            start=(cb == 0),
                            stop=(cb == CB - 1),
                        )
                    nc.vector.tensor_scalar_add(
                        qkT[:, co, tn * 512 : (tn + 1) * 512],
                        ps[:],
                        bqk[:, co : co + 1],
                    )

        # ---- scope 2: phase C (attention per head) ---------------------------
        with ExitStack() as s2:
            p_p = s2.enter_context(tc.tile_pool(name="pp", bufs=20))
            p_small = s2.enter_context(tc.tile_pool(name="small", bufs=4))

            for h in range(HG):
                poff = 64 * (h % 2)
                qh = qkT[poff : poff + 64, h // 2, :]
                kh = qkT[poff : poff + 64, 4 + h // 2, :]
                for ic in range(4):
                    ptiles = {}
                    jt_max = min(4 * (ic + 1), TT)
                    for jt in range(jt_max):
                        ps = ps_sc.tile([128, 512], f32, tag="sc")
                        nc.tensor.matmul(
                            ps[:],
                            kh[:, jt * 128 : (jt + 1) * 128],
                            qh[:, ic * 512 : (ic + 1) * 512],
                            start=True,
                            stop=True,
                        )
                        pt = p_p.tile([128, 512], f16, tag="p")
                        nc.scalar.activation(pt[:], ps[:], Exp, scale=1.0 / np.sqrt(HD))
                        if jt >= 4 * ic:  # chunk contains/behind the diagonal
                            nc.vector.tensor_mul(pt[:], pt[:], masks[:, jt % 4, :])
                        ptiles[jt] = pt
                    for it in range(4 * ic, min(4 * ic + 4, TT)):
                        up = ps_u.tile([128, HD + 1], f32, tag="u")
                        for jt in range(it + 1):
                            nc.tensor.matmul(
                                up[:],
                                ptiles[jt][:, (it % 4) * 128 : (it % 4 + 1) * 128],
                                vv[:, jt, h, :],
                                start=(jt == 0),
                                stop=(jt == it),
                            )
                        rc = p_small.tile([128, 1], f32, tag="recip")
                        nc.vector.reciprocal(rc[:], up[:, HD : HD + 1])
                        nc.vector.tensor_mul(
                            yy[:, it, h * HD : (h + 1) * HD],
                            up[:, 0:HD],
                            rc[:, 0:1].to_broadcast([128, HD]),
                        )

        # ---- scope 3: phase D (yT + projection) ------------------------------
        with ExitStack() as s3:
            p_yT = s3.enter_context(tc.tile_pool(name="yT", bufs=1))
            p_wp = s3.enter_context(tc.tile_pool(name="wp", bufs=1))
            p_ost = s3.enter_context(tc.tile_pool(name="ost", bufs=4))

            yT = p_yT.tile([128, DB, T], f32r)
            for tt in range(TT):
                for db in range(DB):
                    pst = ps_tr.tile([128, 128], f16, tag="tr")
                    nc.tensor.transpose(
                        pst[:], yy[:, tt, db * 128 : (db + 1) * 128], id16[:]
                    )
                    nc.vector.tensor_copy(yT[:, db, tt * 128 : (tt + 1) * 128], pst[:])

            wp_r = p_wp.tile([128, DB, C], f32r)
            for db in range(DB):
                wp_t = p_ost.tile([128, C], f32, tag="wpstage")
                nc.sync.dma_start(wp_t[:], wp_ap[db * 128 : (db + 1) * 128, :])
                nc.vector.tensor_copy(wp_r[:, db, :], wp_t[:])

            for co in range(CB):
                for tn in range(4):
                    ps = ps_mm.tile([128, 512], f32, tag="mm")
                    for db in range(DB):
                        nc.tensor.matmul(
                            ps[:],
                            wp_r[:, db, co * 128 : (co + 1) * 128],
                            yT[:, db, tn * 512 : (tn + 1) * 512],
                            start=(db == 0),
                            stop=(db == DB - 1),
                        )
                    ot = p_ost.tile([128, 512], f32)
                    nc.vector.tensor_copy(ot[:], ps[:])
                    nc.sync.dma_start(
                        outT_ap[co * 128 : (co + 1) * 128, tn * 512 : (tn + 1) * 512],
                        ot[:],
                    )


def _build_program():
    nc = bacc.Bacc("TRN2", target_bir_lowering=False, debug=False, num_devices=N_CORES)
    aps = {
        "x": nc.dram_tensor("x", [T, C], f32, kind="ExternalInput").ap(),
        "wqk": nc.dram_tensor("wqk", [CB, C, 128], f32, kind="ExternalInput").ap(),
        "wv": nc.dram_tensor("wv", [C, CL], f32, kind="ExternalInput").ap(),
        "wp": nc.dram_tensor("wp", [CL, C], f32, kind="ExternalInput").ap(),
        "bqk": nc.dram_tensor("bqk", [CB, 128], f32, kind="ExternalInput").ap(),
        "masks": nc.dram_tensor("masks", [128, 4, 512], f16, kind="ExternalInput").ap(),
        "id32": nc.dram_tensor("id32", [128, 128], f32, kind="ExternalInput").ap(),
        "id16": nc.dram_tensor("id16", [128, 128], f16, kind="ExternalInput").ap(),
        "outT": nc.dram_tensor("outT", [C, T], f32, kind="ExternalOutput").ap(),
    }
    with tile.TileContext(nc) as tc:
        _emit(tc, aps)
    nc.compile()
    return nc


def get_program():
    if "nc" not in _PROG_CACHE:
        _PROG_CACHE["nc"] = _build_program()
    return _PROG_CACHE["nc"]


def _host_consts():
    j = np.arange(128)[:, None]
    i = np.arange(512)[None, :]
    masks = np.zeros((128, 4, 512), np.float16)
    for m in range(4):
        masks[:, m, :] = (j <= i - 128 * m).astype(np.float16)
    id32 = np.eye(128, dtype=np.float32)
    id16 = np.eye(128, dtype=np.float16)
    return masks, id32, id16


def make_in_maps(x, W_attn, b_attn):
    """Build the 8 per-core input maps. Core index = 2*batch + head_group."""
    masks, id32, id16 = _host_consts()
    in_maps = []
    for core in range(N_CORES):
        b = core // 2
        g = core % 2
        wq = W_attn[:, g * CL : (g + 1) * CL]
        wk = W_attn[:, C + g * CL : C + (g + 1) * CL]
        wqk = np.stack(
            [wq[:, i * 128 : (i + 1) * 128] for i in range(4)]
            + [wk[:, i * 128 : (i + 1) * 128] for i in range(4)],
            axis=0,
        )  # [8, C, 128]
        wv = W_attn[:, 2 * C + g * CL : 2 * C + (g + 1) * CL]
        bqk = np.concatenate(
            [b_attn[g * CL : (g + 1) * CL], b_attn[C + g * CL : C + (g + 1) * CL]]
        ).reshape(CB, 128)
        in_maps.append(
            {
                "x": np.ascontiguousarray(x[b]),
                "wqk": np.ascontiguousarray(wqk),
                "wv": np.ascontiguousarray(wv),
                "bqk": np.ascontiguousarray(bqk),
                "masks": masks,
                "id32": id32,
                "id16": id16,
            }
        )
    return in_maps


def add_wp(in_maps, W_proj):
    for core, m in enumerate(in_maps):
        g = core % 2
        m["wp"] = np.ascontiguousarray(W_proj[g * CL : (g + 1) * CL, :])
    return in_maps


def run(x, W_attn, b_attn, W_proj, b_proj, trace=False):
    nc = get_program()
    in_maps = add_wp(make_in_maps(x, W_attn, b_attn), W_proj)
    res = bass_utils.run_bass_kernel_spmd(
        nc, in_maps, core_ids=list(range(N_CORES)), trace=trace
    )
    # combine: out[b] = sum_g outT_{2b+g}^T + (bv_g @ Wp_g summed) + b_proj
    corr = b_proj.astype(np.float64).copy()
    for g in range(2):
        bv_g = b_attn[2 * C + g * CL : 2 * C + (g + 1) * CL]
        corr += bv_g.astype(np.float64) @ W_proj[g * CL : (g + 1) * CL, :].astype(
            np.float64
        )
    out = np.empty((B, T, C), np.float32)
    for b in range(B):
        acc = (
            res.results[2 * b]["outT"].T.astype(np.float64)
            + res.results[2 * b + 1]["outT"].T.astype(np.float64)
            + corr
        )
        out[b] = acc.astype(np.float32)
    return out, res


def kernel(x, W_attn, b_attn, W_proj, b_proj):
    x = np.asarray(x, np.float32)
    W_attn = np.asarray(W_attn, np.float32)
    b_attn = np.asarray(b_attn, np.float32)
    W_proj = np.asarray(W_proj, np.float32)
    b_proj = np.asarray(b_proj, np.float32)
    out, _ = run(x, W_attn, b_attn, W_proj, b_proj)
    return out
